# revision 1
# baseline (speedup 1.0000x reference)
"""Trainium2 Bass kernel for a 4-layer GIN GNN (nn_ClassicGNN).

Strategy (graph/data parallel over 8 NeuronCores, dst-sharded):
  - Nodes are block-sharded: core c owns global nodes [c*NPC, (c+1)*NPC),
    padded to PADN = T*128 locally (fake nodes stay zero).
  - Node features live on-chip feature-major: x^T [D=128 partitions, PADN].
  - Per layer: AllGather row-major features into a DRAM buffer x_full
    [NCORES*PADN, D]; edges (pre-sorted by destination on the host and
    packed into 128-edge chunks per 128-node destination tile) drive
    indirect-DMA row gathers; a one-hot slot matrix M (built on the vector
    engine with is_equal against an iota) turns segment-sum into PSUM
    matmul accumulation: agg^T = sum_k G_k^T @ M_k.
  - h = (1+eps)*x + agg fuses as one scalar_tensor_tensor op; the MLP runs
    with W1/W2 as stationary operands (no transposes); BatchNorm statistics
    are free-dim reductions + a [128,2] AllReduce; BN apply + leaky ReLU
    are two exact elementwise passes.

kernel(**inputs) takes the full-size inputs and returns the full [N, D]
output, distributing across 8 cores internally.
"""

import math

import numpy as np

NCORES = 8
P = 128
D = 128
L = 4
NEG_SLOPE = 0.01
BN_EPS = 1e-5

# x_full / gathered-feature dtype: "float32" (exact) or "bfloat16" (halves
# gather DMA traffic and runs the aggregation matmuls at full PE rate).
DT_X_NAME = "bfloat16"


# ---------------------------------------------------------------------------
# Walrus codegen only encodes a single sync wait per instruction; the Tile
# scheduler sometimes attaches more (DMA lane-reuse + data dep, end-of-kernel
# drains). Hoist extras onto same-engine single-wait NoOps placed immediately
# before the instruction (engine streams execute in order, so AND semantics
# are preserved).
# ---------------------------------------------------------------------------
def _legalize_waits(nc, mybir, max_waits=1):
    n_fixed = 0
    for fn in nc.m.functions:
        for bb in fn.blocks:
            insts = bb.instructions
            out = []
            changed = False
            for inst in insts:
                si = inst.sync_info
                if si is not None and len(si.on_wait) > max_waits:
                    waits = list(si.on_wait)
                    keep = waits[len(waits) - max_waits:]
                    hoist = waits[: len(waits) - max_waits]
                    for w in hoist:
                        nop = mybir.InstNoOp(
                            name=nc.get_next_instruction_name(),
                            ins=[],
                            outs=[],
                            engine=inst.engine,
                            debug=inst.debug,
                        )
                        nop.sync_info = mybir.SyncInfo(on_wait=[w], on_update=[])
                        out.append(nop)
                    inst.sync_info = mybir.SyncInfo(
                        on_wait=keep, on_update=list(si.on_update)
                    )
                    n_fixed += 1
                    changed = True
                out.append(inst)
            if changed:
                insts[:] = out
    return n_fixed


# ---------------------------------------------------------------------------
# Host-side preprocessing: shard nodes, sort/pack edges by destination tile.
# ---------------------------------------------------------------------------
def _prep(node_deg, edge_index, n_nodes):
    N = n_nodes
    E = edge_index.shape[1]
    NPC = N // NCORES
    T = math.ceil((NPC + 1) / P)  # always >= 1 fake (zero) row per core
    PADN = T * P
    NW = NCORES // 2              # int16 source windows of 2*PADN rows
    W = 2 * PADN
    assert W <= 32767

    src = edge_index[0].astype(np.int64)
    dst = edge_index[1].astype(np.int64)
    core = dst // NPC
    ld = dst % NPC
    srow = (src // NPC) * PADN + (src % NPC)
    t_of = ld // P
    slot = ld % P
    w_of = srow // W
    lidx = srow - w_of * W

    key = ((core * T + t_of) * NW + w_of)
    order = np.argsort(key, kind="stable")
    ks = key[order]
    lidx_s = lidx[order].astype(np.int16)
    slot_s = slot[order].astype(np.float32)

    counts = np.bincount(key, minlength=NCORES * T * NW)
    starts = np.zeros(NCORES * T * NW + 1, np.int64)
    starts[1:] = np.cumsum(counts)
    rank = np.arange(E, dtype=np.int64) - starts[ks]

    cnt = counts.reshape(NCORES, T, NW)
    K_tw = ((cnt.max(axis=0) + P - 1) // P).astype(np.int64)   # [T, NW]
    # per-call descriptor count: round the max core count to the 16-index
    # wrapping granularity (not 128) so Q7 skips most pad descriptors
    NV16 = (np.maximum(cnt.max(axis=0), 1) + 15) // 16 * 16    # [T, NW]
    CPT = K_tw.sum(axis=1).astype(np.int64)                    # chunks per tile
    off_tw = np.zeros(T * NW + 1, np.int64)
    off_tw[1:] = np.cumsum(K_tw.ravel())
    off_tw = off_tw[:-1].reshape(T, NW)                        # global chunk col
    off_t = np.zeros(T + 1, np.int64)
    off_t[1:] = np.cumsum(CPT)
    CH = int(off_t[-1])

    # pads are negative indices (the Q7 loop skips them without emitting a
    # descriptor) and carry slot=-1 so the one-hot matrix zeroes whatever
    # stale data sits in the skipped G positions. Each (t,w) call keeps at
    # least one valid index (row 0) so the gather is never empty.
    idx16 = np.zeros((NCORES, P, CH), np.int16)
    slot_arr = np.full((NCORES, P, CH), -1.0, np.float32)
    tw = ks % (T * NW)
    col = off_tw.ravel()[tw] + rank // P
    part = rank % P
    idx16[ks // (T * NW), part, col] = lidx_s
    slot_arr[ks // (T * NW), part, col] = slot_s
    # empty (core,t,w) groups with K_tw>0 get one valid dummy index
    nvalid = cnt.copy()                     # [NCORES, T, NW]
    for c in range(NCORES):
        empty = (cnt[c] == 0) & (K_tw > 0)
        for t0, w0 in zip(*np.nonzero(empty)):
            idx16[c, 0, off_tw[t0, w0]] = 0
            nvalid[c, t0, w0] = 1

    # dma_gather wrapped index layout: flat index i -> partition i%16,
    # column i//16, replicated over the 8 gpsimd cores (16-partition blocks).
    a = idx16.reshape(NCORES, 8, 16, CH)        # p = s*16 + r -> (s, r)
    wr = a.transpose(0, 2, 3, 1).reshape(NCORES, 16, CH * 8)
    widx = np.ascontiguousarray(np.tile(wr, (1, 8, 1)))  # [NC, 128, CH*8]

    # per-tile degree-index table [NCORES, P, T]; fake slots -> row 64 (zeros)
    deg_idx = np.full((NCORES, P, T), 64, np.int32)
    n = np.arange(NPC, dtype=np.int64)
    deg_idx[:, n % P, n // P] = node_deg.reshape(NCORES, NPC).astype(np.int32)

    # layer-0 aggregation is a degree histogram: agg0[n] = sum_k C[n,k] embed[k]
    # (x0 = embed[node_deg] has only 64 distinct rows). C is exact small ints.
    sdeg = node_deg.astype(np.int64)[src]
    Cfull = np.bincount(dst * 64 + sdeg, minlength=N * 64).reshape(N, 64)
    ct = np.zeros((NCORES, 64, PADN), np.float32)
    for c in range(NCORES):
        ct[c, :, :NPC] = Cfull[c * NPC:(c + 1) * NPC].T

    # per-core valid-count table, one int32 per emitted gather call,
    # in (t, w) emission order
    call_counts = []
    for t0 in range(T):
        for w0 in range(NW):
            if K_tw[t0, w0] > 0:
                call_counts.append(nvalid[:, t0, w0])
    ncalls = len(call_counts)
    cnt_tab = np.zeros((NCORES, 1, max(ncalls, 1)), np.int32)
    if ncalls:
        cnt_tab[:, 0, :] = np.array(call_counts, np.int32).T

    meta = dict(N=N, NPC=NPC, T=T, PADN=PADN, CH=CH, NW=NW, W=W,
                NCALLS=ncalls,
                K_tw=[[int(k) for k in row] for row in K_tw],
                NV16=[[int(v) for v in row] for row in NV16],
                CPT=[int(c) for c in CPT],
                off_tw=[[int(o) for o in row] for row in off_tw],
                off_t=[int(o) for o in off_t])
    return meta, widx, slot_arr, deg_idx, ct, cnt_tab


# ---------------------------------------------------------------------------
# Bass/Tile program
# ---------------------------------------------------------------------------
def _build(meta, eps_vals, dt_x_name, legalize=True, debug_taps=False):
    import concourse.bass as bass
    import concourse.bacc as bacc
    import concourse.mybir as mybir
    import concourse.tile as tile
    from concourse.bass import IndirectOffsetOnAxis, ts
    from concourse.masks import make_identity

    # The end-of-kernel semaphore RANGE_CLEAR chokes walrus codegen ("ISA
    # wrong length") when the range is large; clear in small chunks instead.
    if not getattr(bass.Bass, "_cafs_chunked", False):
        _orig_cafs = bass.Bass.clear_and_free_semaphores

        def _chunked(self, sems, _orig=_orig_cafs):
            sems = list(sems)
            for i in range(0, len(sems), 8):
                _orig(self, sems[i:i + 8])

        bass.Bass.clear_and_free_semaphores = _chunked
        bass.Bass._cafs_chunked = True

    f32 = mybir.dt.float32
    dt_x = getattr(mybir.dt, dt_x_name)
    T, PADN, CH, NPC = meta["T"], meta["PADN"], meta["CH"], meta["NPC"]
    NW, W = meta["NW"], meta["W"]
    NCALLS = max(meta["NCALLS"], 1)
    K_tw, CPT, off_tw, off_t = meta["K_tw"], meta["CPT"], meta["off_tw"], meta["off_t"]
    NV16 = meta["NV16"]
    N = meta["N"]
    XROWS = NCORES * PADN
    CPT_max = max(CPT)
    group = [list(range(NCORES))]

    nc = bacc.Bacc("TRN2", num_devices=NCORES, debug=False,
                   dynamic_dma_scratch_size=32768)

    # --- I/O ---
    embed_d = nc.dram_tensor("embed", [65, D], f32, kind="ExternalInput")
    w1_d = nc.dram_tensor("w1", [L * D, D], f32, kind="ExternalInput")
    w2_d = nc.dram_tensor("w2", [L * D, D], f32, kind="ExternalInput")
    b1_d = nc.dram_tensor("b1t", [P, L], f32, kind="ExternalInput")
    b2_d = nc.dram_tensor("b2t", [P, L], f32, kind="ExternalInput")
    gm_d = nc.dram_tensor("gammat", [P, L - 1], f32, kind="ExternalInput")
    bt_d = nc.dram_tensor("betat", [P, L - 1], f32, kind="ExternalInput")
    idx_d = nc.dram_tensor("idx", [P, CH * 8], mybir.dt.int16, kind="ExternalInput")
    slot_d = nc.dram_tensor("slotv", [P, CH], f32, kind="ExternalInput")
    degidx_d = nc.dram_tensor("degidx", [P, T], mybir.dt.int32, kind="ExternalInput")
    ct_d = nc.dram_tensor("ct", [64, PADN], f32, kind="ExternalInput")
    cnt_d = nc.dram_tensor("cnt", [1, NCALLS], mybir.dt.int32, kind="ExternalInput")
    out_d = nc.dram_tensor("out", [P, PADN], f32, kind="ExternalOutput")
    if debug_taps:
        dbg_xb = nc.dram_tensor("dbg_xb", [P, PADN], f32, kind="ExternalOutput")
        dbg_st = nc.dram_tensor("dbg_st", [P, 6], f32, kind="ExternalOutput")
        dbg_xa = nc.dram_tensor("dbg_xa", [P, PADN], f32, kind="ExternalOutput")
        dbg_h = nc.dram_tensor("dbg_h", [P, PADN], f32, kind="ExternalOutput")
        dbg_z1 = nc.dram_tensor("dbg_z1", [P, PADN], f32, kind="ExternalOutput")

    with tile.TileContext(nc) as tc:
        with tc.tile_pool(name="persist", bufs=1) as pp, \
             tc.tile_pool(name="gpool", bufs=3) as gp, \
             tc.tile_pool(name="mpool", bufs=3) as mp, \
             tc.tile_pool(name="ipool", bufs=3) as ip, \
             tc.tile_pool(name="small", bufs=2) as sp, \
             tc.tile_pool(name="rowp", bufs=4) as rp, \
             tc.tile_pool(name="agg_ps", bufs=2, space="PSUM") as aggp, \
             tc.tile_pool(name="mlp_ps", bufs=2, space="PSUM") as mlpp, \
             tc.tile_pool(name="tr_ps", bufs=2, space="PSUM") as trp, \
             tc.tile_pool(name="dram", bufs=1, space="DRAM") as dp:

            # --- persistent SBUF state ---
            xA = pp.tile([P, PADN], f32)          # x^T (current layer input)
            xB = pp.tile([P, PADN], f32)          # z2^T (layer output)
            slot_res = pp.tile([P, CH], dt_x)
            deg_res = pp.tile([P, T], mybir.dt.int32)
            w1_sb = pp.tile([P, L * D], f32)
            w2_sb = pp.tile([P, L * D], f32)
            b1_sb = pp.tile([P, L], f32)
            b2_sb = pp.tile([P, L], f32)
            gm_sb = pp.tile([P, L - 1], f32)
            bt_sb = pp.tile([P, L - 1], f32)
            ident = pp.tile([P, P], f32)
            iota_x = pp.tile([P, P], dt_x)
            iota_i = pp.tile([P, P], mybir.dt.int32)
            embed_x = pp.tile([64, D], dt_x)
            cnt_res = pp.tile([1, NCALLS], mybir.dt.int32)
            g_bufs = [pp.tile([P, CPT_max * D], dt_x, name=f"gbuf{i}")
                      for i in range(3)]
            sum_cols = pp.tile([P, T], f32)
            sq_cols = pp.tile([P, T], f32)
            ssum = pp.tile([P, 2], f32)
            gstat = pp.tile([P, 2], f32)
            bn_sc = pp.tile([P, 8], f32)          # scratch columns for BN math

            # --- DRAM buffers ---
            xin = dp.tile([PADN, D], dt_x)
            x_fulls = [dp.tile([XROWS, D], dt_x, addr_space="Shared",
                               name=f"x_full{i}") for i in range(L)]
            st_in = dp.tile([P, 2], f32)
            st_outs = [dp.tile([P, 2], f32, addr_space="Shared",
                               name=f"st_out{i}") for i in range(L - 1)]

            # --- load constants ---
            if dt_x == f32:
                nc.sync.dma_start(out=slot_res[:], in_=slot_d[:])
            else:
                nc.gpsimd.dma_start(out=slot_res[:], in_=slot_d[:])  # f32->bf16 cast
            nc.sync.dma_start(out=deg_res[:], in_=degidx_d[:])
            nc.sync.dma_start(out=w1_sb[:], in_=w1_d[:].rearrange("(l k) m -> k l m", k=P))
            nc.sync.dma_start(out=w2_sb[:], in_=w2_d[:].rearrange("(l k) m -> k l m", k=P))
            nc.sync.dma_start(out=b1_sb[:], in_=b1_d[:])
            nc.sync.dma_start(out=b2_sb[:], in_=b2_d[:])
            nc.sync.dma_start(out=gm_sb[:], in_=gm_d[:])
            nc.sync.dma_start(out=bt_sb[:], in_=bt_d[:])
            make_identity(nc, ident[:])
            nc.sync.dma_start(out=cnt_res[:], in_=cnt_d[:])
            nv_reg = nc.gpsimd.alloc_register("nv")
            for gb in g_bufs:
                nc.vector.memset(gb[:], 0.0)
            g_rot = [0]
            embed_f = pp.tile([64, D], f32)
            nc.sync.dma_start(out=embed_f[:], in_=embed_d[0:64, :])
            nc.vector.tensor_copy(embed_x[:], embed_f[:])
            nc.gpsimd.iota(iota_i[:], pattern=[[1, P]], base=0, channel_multiplier=0)
            nc.vector.tensor_copy(iota_x[:], iota_i[:])

            def iota_bc(K):
                a = iota_x[:]
                return bass.AP(a.tensor, a.offset, [a.ap[0], [0, K], a.ap[1]])

            # --- prologue: x0 = embed[deg], write xin rows + xA (transposed) ---
            for t in range(T):
                rows = rp.tile([P, D], f32, name="rows0")
                nc.gpsimd.indirect_dma_start(
                    out=rows[:], out_offset=None,
                    in_=embed_d[:],
                    in_offset=IndirectOffsetOnAxis(ap=deg_res[:, t:t + 1], axis=0),
                )
                tr = trp.tile([P, P], f32, space="PSUM", name="trt")
                nc.tensor.transpose(out=tr[:], in_=rows[:], identity=ident[:])
                nc.vector.tensor_copy(xA[:, ts(t, P)], tr[:])

            for lyr in range(L):
                # AllGather features for this layer (layer 0 needs none: its
                # aggregation is the C-matrix matmul, not an edge gather)
                x_full = x_fulls[lyr]
                if lyr > 0:
                    nc.gpsimd.collective_compute(
                        "AllGather", mybir.AluOpType.bypass,
                        replica_groups=group,
                        ins=[xin.opt()], outs=[x_full.opt()],
                    )

                last = lyr == L - 1
                epsf = float(1.0 + eps_vals[lyr])
                call_no = [[-1] * NW for _ in range(T)]
                _cn = 0
                for _t in range(T):
                    for _w in range(NW):
                        if K_tw[_t][_w] > 0:
                            call_no[_t][_w] = _cn
                            _cn += 1
                for t in range(T):
                    K = CPT[t]
                    o = off_t[t]
                    h = sp.tile([P, P], f32, name="h")
                    if lyr == 0:
                        # agg0 = (C @ embed)^T: one K=64 matmul per tile
                        ctt = ip.tile([64, P], dt_x, name="ctt")
                        if dt_x == f32:
                            nc.sync.dma_start(out=ctt[:], in_=ct_d[:, ts(t, P)])
                        else:
                            nc.gpsimd.dma_start(out=ctt[:], in_=ct_d[:, ts(t, P)])
                        agg = aggp.tile([P, P], f32, space="PSUM", name="agg")
                        nc.tensor.matmul(out=agg[:], lhsT=embed_x[:], rhs=ctt[:],
                                         start=True, stop=True)
                        nc.vector.scalar_tensor_tensor(
                            out=h[:], in0=xA[:, ts(t, P)], scalar=epsf, in1=agg[:],
                            op0=mybir.AluOpType.mult, op1=mybir.AluOpType.add,
                        )
                        _mlp_tail_marker = True
                    elif K == 0:
                        # all-fake tile: no incoming edges, agg == 0
                        nc.vector.tensor_scalar_mul(h[:], xA[:, ts(t, P)], epsf)
                    else:
                        G = g_bufs[g_rot[0] % 3]
                        g_rot[0] += 1
                        it = ip.tile([P, CPT_max * 8], mybir.dt.int16, name="it")
                        nc.sync.dma_start(out=it[:, :K * 8],
                                          in_=idx_d[:, o * 8:(o + K) * 8])
                        for w in range(NW):
                            Kw = K_tw[t][w]
                            if Kw == 0:
                                continue
                            c0 = off_tw[t][w] - o
                            nc.gpsimd.dma_gather(
                                out_ap=G[:, c0 * D:(c0 + Kw) * D].rearrange(
                                    "p (k j) -> p k j", j=D),
                                in_ap=x_full[w * W:(w + 1) * W, :],
                                idxs_ap=it[:, c0 * 8:c0 * 8 + NV16[t][w] // 16],
                                num_idxs=NV16[t][w],
                                num_idxs_reg=NV16[t][w],
                                elem_size=D,
                            )
                        M = mp.tile([P, CPT_max * D], dt_x, name="M")
                        nc.vector.tensor_tensor(
                            out=M[:, :K * D].rearrange("p (k j) -> p k j", j=P),
                            in0=slot_res[:, o:o + K].to_broadcast([P, K, P]),
                            in1=iota_bc(K),
                            op=mybir.AluOpType.is_equal,
                        )
                        agg = aggp.tile([P, P], f32, space="PSUM", name="agg")
                        for k in range(K):
                            nc.tensor.matmul(
                                out=agg[:], lhsT=G[:, ts(k, D)], rhs=M[:, ts(k, D)],
                                start=(k == 0), stop=(k == K - 1),
                            )
                        # h = (1+eps)*x + agg
                        nc.vector.scalar_tensor_tensor(
                            out=h[:], in0=xA[:, ts(t, P)], scalar=epsf, in1=agg[:],
                            op0=mybir.AluOpType.mult, op1=mybir.AluOpType.add,
                        )
                    if debug_taps and lyr == 0:
                        nc.sync.dma_start(out=dbg_h[:, ts(t, P)], in_=h[:])
                    z1p = mlpp.tile([P, P], f32, space="PSUM", name="z1p")
                    nc.tensor.matmul(out=z1p[:], lhsT=w1_sb[:, ts(lyr, D)], rhs=h[:],
                                     start=True, stop=True)
                    # z1 = lrelu(z1p + b1)
                    t1 = sp.tile([P, P], f32, name="t1")
                    nc.scalar.activation(
                        out=t1[:], in_=z1p[:],
                        func=mybir.ActivationFunctionType.Identity,
                        bias=b1_sb[:, lyr:lyr + 1], scale=1.0)
                    z1 = sp.tile([P, P], f32, name="z1")
                    nc.vector.scalar_tensor_tensor(
                        out=z1[:], in0=t1[:], scalar=NEG_SLOPE, in1=t1[:],
                        op0=mybir.AluOpType.mult, op1=mybir.AluOpType.max,
                    )
                    if debug_taps and lyr == 0:
                        nc.sync.dma_start(out=dbg_z1[:, ts(t, P)], in_=z1[:])
                    z2p = mlpp.tile([P, P], f32, space="PSUM", name="z2p")
                    nc.tensor.matmul(out=z2p[:], lhsT=w2_sb[:, ts(lyr, D)], rhs=z1[:],
                                     start=True, stop=True)
                    # xB_tile = z2p + b2 (+ per-tile stats sum)
                    if not last:
                        nc.scalar.activation(
                            out=xB[:, ts(t, P)], in_=z2p[:],
                            func=mybir.ActivationFunctionType.Identity,
                            bias=b2_sb[:, lyr:lyr + 1], scale=1.0,
                            accum_out=sum_cols[:, t:t + 1])
                        sq = sp.tile([P, P], f32, name="sq")
                        nc.vector.tensor_tensor(
                            out=sq[:], in0=xB[:, ts(t, P)], in1=xB[:, ts(t, P)],
                            op=mybir.AluOpType.mult)
                        nc.vector.reduce_sum(out=sq_cols[:, t:t + 1], in_=sq[:],
                                             axis=mybir.AxisListType.X)
                    else:
                        nc.scalar.activation(
                            out=xB[:, ts(t, P)], in_=z2p[:],
                            func=mybir.ActivationFunctionType.Identity,
                            bias=b2_sb[:, lyr:lyr + 1], scale=1.0)

                if last:
                    nc.sync.dma_start(out=out_d[:], in_=xB[:])
                    continue

                # fake-node columns must not pollute the batch statistics
                if PADN > NPC:
                    nc.gpsimd.memset(xB[:, NPC:PADN], 0.0)
                    lt = T - 1
                    nc.scalar.activation(
                        out=xB[:, ts(lt, P)], in_=xB[:, ts(lt, P)],
                        func=mybir.ActivationFunctionType.Identity,
                        bias=0.0, scale=1.0, accum_out=sum_cols[:, lt:lt + 1])
                    sq = sp.tile([P, P], f32, name="sqf")
                    nc.vector.tensor_tensor(
                        out=sq[:], in0=xB[:, ts(lt, P)], in1=xB[:, ts(lt, P)],
                        op=mybir.AluOpType.mult)
                    nc.vector.reduce_sum(out=sq_cols[:, lt:lt + 1], in_=sq[:],
                                         axis=mybir.AxisListType.X)

                if debug_taps and lyr == 0:
                    nc.sync.dma_start(out=dbg_xb[:], in_=xB[:])
                # global BN statistics
                nc.vector.reduce_sum(out=ssum[:, 0:1], in_=sum_cols[:],
                                     axis=mybir.AxisListType.X)
                nc.vector.reduce_sum(out=ssum[:, 1:2], in_=sq_cols[:],
                                     axis=mybir.AxisListType.X)
                nc.sync.dma_start(out=st_in[:], in_=ssum[:])
                st_out = st_outs[lyr]
                nc.gpsimd.collective_compute(
                    "AllReduce", mybir.AluOpType.add,
                    replica_groups=group,
                    ins=[st_in.opt()], outs=[st_out.opt()],
                )
                nc.sync.dma_start(out=gstat[:], in_=st_out[:])
                mu = bn_sc[:, 0:1]; ex2 = bn_sc[:, 1:2]; musq = bn_sc[:, 2:3]
                var = bn_sc[:, 3:4]; std = bn_sc[:, 4:5]; rstd = bn_sc[:, 5:6]
                scl = bn_sc[:, 6:7]; sft = bn_sc[:, 7:8]
                nc.vector.tensor_scalar_mul(mu, gstat[:, 0:1], 1.0 / N)
                nc.vector.tensor_scalar_mul(ex2, gstat[:, 1:2], 1.0 / N)
                nc.vector.tensor_tensor(out=musq, in0=mu, in1=mu,
                                        op=mybir.AluOpType.mult)
                nc.vector.tensor_sub(var, ex2, musq)
                nc.vector.tensor_scalar_add(var, var, BN_EPS)
                nc.scalar.activation(out=std, in_=var,
                                     func=mybir.ActivationFunctionType.Sqrt,
                                     bias=0.0, scale=1.0)
                nc.vector.reciprocal(rstd, std)
                nc.vector.tensor_tensor(out=scl, in0=rstd,
                                        in1=gm_sb[:, lyr:lyr + 1],
                                        op=mybir.AluOpType.mult)
                nc.vector.tensor_tensor(out=musq, in0=mu, in1=scl,
                                        op=mybir.AluOpType.mult)
                nc.vector.tensor_sub(sft, bt_sb[:, lyr:lyr + 1], musq)
                # xA = lrelu(xB*scl + sft); then re-zero fake columns
                nc.scalar.activation(
                    out=xB[:], in_=xB[:],
                    func=mybir.ActivationFunctionType.Identity,
                    bias=sft, scale=scl)
                nc.vector.scalar_tensor_tensor(
                    out=xA[:], in0=xB[:], scalar=NEG_SLOPE, in1=xB[:],
                    op0=mybir.AluOpType.mult, op1=mybir.AluOpType.max,
                )
                if PADN > NPC:
                    nc.gpsimd.memset(xA[:, NPC:PADN], 0.0)
                if debug_taps and lyr == 0:
                    nc.sync.dma_start(out=dbg_st[:, 0:2], in_=ssum[:])
                    nc.sync.dma_start(out=dbg_st[:, 2:4], in_=gstat[:])
                    nc.sync.dma_start(out=dbg_st[:, 4:5], in_=bn_sc[:, 6:7])
                    nc.sync.dma_start(out=dbg_st[:, 5:6], in_=bn_sc[:, 7:8])
                    nc.sync.dma_start(out=dbg_xa[:], in_=xA[:])

                # epilogue: transpose back to rows, refresh xin for next AG
                for t in range(T):
                    tr = trp.tile([P, P], f32, space="PSUM", name="trt")
                    nc.tensor.transpose(out=tr[:], in_=xA[:, ts(t, P)],
                                        identity=ident[:])
                    rows = rp.tile([P, D], dt_x, name="rowse")
                    nc.vector.tensor_copy(rows[:], tr[:])
                    nc.sync.dma_start(out=xin[ts(t, P), :], in_=rows[:])

    nc.compile()
    if legalize:
        import concourse.mybir as mybir2
        _legalize_waits(nc, mybir2)
    return nc


def _host_inputs(inputs, meta, idx_arr, slot_arr, deg_idx, ct, cnt_tab):
    embed = np.zeros((65, D), np.float32)
    embed[:64] = np.asarray(inputs["embed_deg"], np.float32)
    W1 = np.asarray(inputs["W1"], np.float32).reshape(L * D, D)
    W2 = np.asarray(inputs["W2"], np.float32).reshape(L * D, D)
    b1t = np.ascontiguousarray(np.asarray(inputs["b1"], np.float32).T)  # [D, L]
    b2t = np.ascontiguousarray(np.asarray(inputs["b2"], np.float32).T)
    gmt = np.ascontiguousarray(np.asarray(inputs["bn_gamma"], np.float32).T)
    btt = np.ascontiguousarray(np.asarray(inputs["bn_beta"], np.float32).T)
    in_maps = []
    for c in range(NCORES):
        in_maps.append(dict(
            embed=embed, w1=W1, w2=W2, b1t=b1t, b2t=b2t, gammat=gmt, betat=btt,
            idx=idx_arr[c], slotv=slot_arr[c], degidx=deg_idx[c], ct=ct[c],
            cnt=cnt_tab[c],
        ))
    return in_maps


def run_gnn(inputs, n_nodes, dt_x_name=DT_X_NAME, trace=False, tmpdir=None):
    """Build + run the SPMD kernel; returns ([N, D] float32, exec_time_ns)."""
    from concourse.bass_utils import run_bass_kernel_spmd

    node_deg = np.asarray(inputs["node_deg"])
    edge_index = np.asarray(inputs["edge_index"])
    meta, idx_arr, slot_arr, deg_idx, ct, cnt_tab = _prep(node_deg, edge_index, n_nodes)
    eps_vals = [float(e) for e in np.asarray(inputs["eps"], np.float32)]
    nc = _build(meta, eps_vals, dt_x_name)
    in_maps = _host_inputs(inputs, meta, idx_arr, slot_arr, deg_idx, ct, cnt_tab)
    res = run_bass_kernel_spmd(
        nc, in_maps, list(range(NCORES)), trace=trace, tmpdir=tmpdir)
    NPC = meta["NPC"]
    parts = [res.results[c]["out"][:, :NPC] for c in range(NCORES)]
    out = np.concatenate(parts, axis=1).T.astype(np.float32)
    return np.ascontiguousarray(out), res.exec_time_ns


def kernel(**inputs):
    out, _ = run_gnn(inputs, n_nodes=100000)
    return out



# revision 3
# speedup vs baseline: 2.5072x; 2.5072x over previous
"""Trainium2 Bass kernel for a 4-layer GIN GNN (nn_ClassicGNN).

Strategy (graph/data parallel over 8 NeuronCores, dst-sharded):
  - Nodes are block-sharded: core c owns global nodes [c*NPC, (c+1)*NPC),
    padded to PADN = T*128 locally (fake nodes stay zero).
  - Node features live on-chip feature-major: x^T [D=128 partitions, PADN].
  - Per layer: AllGather row-major features into a DRAM buffer x_full
    [NCORES*PADN, D]; edges (pre-sorted by destination on the host and
    packed into 128-edge chunks per 128-node destination tile) drive
    indirect-DMA row gathers; a one-hot slot matrix M (built on the vector
    engine with is_equal against an iota) turns segment-sum into PSUM
    matmul accumulation: agg^T = sum_k G_k^T @ M_k.
  - h = (1+eps)*x + agg fuses as one scalar_tensor_tensor op; the MLP runs
    with W1/W2 as stationary operands (no transposes); BatchNorm statistics
    are free-dim reductions + a [128,2] AllReduce; BN apply + leaky ReLU
    are two exact elementwise passes.

kernel(**inputs) takes the full-size inputs and returns the full [N, D]
output, distributing across 8 cores internally.
"""

import math

import numpy as np

NCORES = 8
P = 128
D = 128
L = 4
NEG_SLOPE = 0.01
BN_EPS = 1e-5

# x_full / gathered-feature dtype: "float32" (exact) or "bfloat16" (halves
# gather DMA traffic and runs the aggregation matmuls at full PE rate).
DT_X_NAME = "bfloat16"


# ---------------------------------------------------------------------------
# Walrus codegen only encodes a single sync wait per instruction; the Tile
# scheduler sometimes attaches more (DMA lane-reuse + data dep, end-of-kernel
# drains). Hoist extras onto same-engine single-wait NoOps placed immediately
# before the instruction (engine streams execute in order, so AND semantics
# are preserved).
# ---------------------------------------------------------------------------
def _legalize_waits(nc, mybir, max_waits=1):
    n_fixed = 0
    for fn in nc.m.functions:
        for bb in fn.blocks:
            insts = bb.instructions
            out = []
            changed = False
            for inst in insts:
                si = inst.sync_info
                if si is not None and len(si.on_wait) > max_waits:
                    waits = list(si.on_wait)
                    keep = waits[len(waits) - max_waits:]
                    hoist = waits[: len(waits) - max_waits]
                    for w in hoist:
                        nop = mybir.InstNoOp(
                            name=nc.get_next_instruction_name(),
                            ins=[],
                            outs=[],
                            engine=inst.engine,
                            debug=inst.debug,
                        )
                        nop.sync_info = mybir.SyncInfo(on_wait=[w], on_update=[])
                        out.append(nop)
                    inst.sync_info = mybir.SyncInfo(
                        on_wait=keep, on_update=list(si.on_update)
                    )
                    n_fixed += 1
                    changed = True
                out.append(inst)
            if changed:
                insts[:] = out
    return n_fixed


# ---------------------------------------------------------------------------
# Host-side preprocessing: shard nodes, sort/pack edges by destination tile.
# ---------------------------------------------------------------------------
def _prep(node_deg, edge_index, n_nodes):
    N = n_nodes
    E = edge_index.shape[1]
    NPC = N // NCORES
    T = math.ceil((NPC + 1) / P)  # always >= 1 fake (zero) row per core
    PADN = T * P
    NW = NCORES // 2              # int16 source windows of 2*PADN rows
    W = 2 * PADN
    assert W <= 32767

    src = edge_index[0].astype(np.int64)
    dst = edge_index[1].astype(np.int64)
    core = dst // NPC
    ld = dst % NPC
    srow = (src // NPC) * PADN + (src % NPC)
    t_of = ld // P
    slot = ld % P
    w_of = srow // W
    lidx = srow - w_of * W

    key = ((core * T + t_of) * NW + w_of)
    order = np.argsort(key, kind="stable")
    ks = key[order]
    lidx_s = lidx[order].astype(np.int16)
    slot_s = slot[order].astype(np.float32)

    counts = np.bincount(key, minlength=NCORES * T * NW)
    starts = np.zeros(NCORES * T * NW + 1, np.int64)
    starts[1:] = np.cumsum(counts)
    rank = np.arange(E, dtype=np.int64) - starts[ks]

    cnt = counts.reshape(NCORES, T, NW)
    K_tw = ((cnt.max(axis=0) + P - 1) // P).astype(np.int64)   # [T, NW]
    # per-call descriptor count: round the max core count to the 16-index
    # wrapping granularity (not 128) so Q7 skips most pad descriptors
    NV16 = (np.maximum(cnt.max(axis=0), 1) + 15) // 16 * 16    # [T, NW]
    CPT = K_tw.sum(axis=1).astype(np.int64)                    # chunks per tile
    off_tw = np.zeros(T * NW + 1, np.int64)
    off_tw[1:] = np.cumsum(K_tw.ravel())
    off_tw = off_tw[:-1].reshape(T, NW)                        # global chunk col
    off_t = np.zeros(T + 1, np.int64)
    off_t[1:] = np.cumsum(CPT)
    CH = int(off_t[-1])

    # pads are negative indices (the Q7 loop skips them without emitting a
    # descriptor) and carry slot=-1 so the one-hot matrix zeroes whatever
    # stale data sits in the skipped G positions. Each (t,w) call keeps at
    # least one valid index (row 0) so the gather is never empty.
    idx16 = np.zeros((NCORES, P, CH), np.int16)
    slot_arr = np.full((NCORES, P, CH), -1.0, np.float32)
    tw = ks % (T * NW)
    col = off_tw.ravel()[tw] + rank // P
    part = rank % P
    idx16[ks // (T * NW), part, col] = lidx_s
    slot_arr[ks // (T * NW), part, col] = slot_s
    # empty (core,t,w) groups with K_tw>0 get one valid dummy index
    nvalid = cnt.copy()                     # [NCORES, T, NW]
    for c in range(NCORES):
        empty = (cnt[c] == 0) & (K_tw > 0)
        for t0, w0 in zip(*np.nonzero(empty)):
            idx16[c, 0, off_tw[t0, w0]] = 0
            nvalid[c, t0, w0] = 1

    # dma_gather wrapped index layout: flat index i -> partition i%16,
    # column i//16, replicated over the 8 gpsimd cores (16-partition blocks).
    a = idx16.reshape(NCORES, 8, 16, CH)        # p = s*16 + r -> (s, r)
    wr = a.transpose(0, 2, 3, 1).reshape(NCORES, 16, CH * 8)
    widx = np.ascontiguousarray(np.tile(wr, (1, 8, 1)))  # [NC, 128, CH*8]

    # per-tile degree-index table [NCORES, P, T]; fake slots -> row 64 (zeros)
    deg_idx = np.full((NCORES, P, T), 64, np.int32)
    n = np.arange(NPC, dtype=np.int64)
    deg_idx[:, n % P, n // P] = node_deg.reshape(NCORES, NPC).astype(np.int32)

    # layer-0 aggregation is a degree histogram: agg0[n] = sum_k C[n,k] embed[k]
    # (x0 = embed[node_deg] has only 64 distinct rows). C is exact small ints.
    sdeg = node_deg.astype(np.int64)[src]
    Cfull = np.bincount(dst * 64 + sdeg, minlength=N * 64).reshape(N, 64)
    ct = np.zeros((NCORES, 64, PADN), np.float32)
    for c in range(NCORES):
        ct[c, :, :NPC] = Cfull[c * NPC:(c + 1) * NPC].T

    # per-core valid-count table, one int32 per emitted gather call,
    # in (t, w) emission order
    call_counts = []
    for t0 in range(T):
        for w0 in range(NW):
            if K_tw[t0, w0] > 0:
                call_counts.append(nvalid[:, t0, w0])
    ncalls = len(call_counts)
    cnt_tab = np.zeros((NCORES, 1, max(ncalls, 1)), np.int32)
    if ncalls:
        cnt_tab[:, 0, :] = np.array(call_counts, np.int32).T

    meta = dict(N=N, NPC=NPC, T=T, PADN=PADN, CH=CH, NW=NW, W=W,
                NCALLS=ncalls,
                K_tw=[[int(k) for k in row] for row in K_tw],
                NV16=[[int(v) for v in row] for row in NV16],
                CPT=[int(c) for c in CPT],
                off_tw=[[int(o) for o in row] for row in off_tw],
                off_t=[int(o) for o in off_t])
    return meta, widx, slot_arr, deg_idx, ct, cnt_tab


# ---------------------------------------------------------------------------
# Bass/Tile program
# ---------------------------------------------------------------------------
def _build(meta, eps_vals, dt_x_name, legalize=True, debug_taps=False):
    import concourse.bass as bass
    import concourse.bacc as bacc
    import concourse.mybir as mybir
    import concourse.tile as tile
    from concourse.bass import IndirectOffsetOnAxis, ts
    from concourse.masks import make_identity

    # The end-of-kernel semaphore RANGE_CLEAR chokes walrus codegen ("ISA
    # wrong length") when the range is large; clear in small chunks instead.
    if not getattr(bass.Bass, "_cafs_chunked", False):
        _orig_cafs = bass.Bass.clear_and_free_semaphores

        def _chunked(self, sems, _orig=_orig_cafs):
            sems = list(sems)
            for i in range(0, len(sems), 8):
                _orig(self, sems[i:i + 8])

        bass.Bass.clear_and_free_semaphores = _chunked
        bass.Bass._cafs_chunked = True

    f32 = mybir.dt.float32
    dt_x = getattr(mybir.dt, dt_x_name)
    T, PADN, CH, NPC = meta["T"], meta["PADN"], meta["CH"], meta["NPC"]
    NW, W = meta["NW"], meta["W"]
    NCALLS = max(meta["NCALLS"], 1)
    K_tw, CPT, off_tw, off_t = meta["K_tw"], meta["CPT"], meta["off_tw"], meta["off_t"]
    NV16 = meta["NV16"]
    N = meta["N"]
    XROWS = NCORES * PADN
    CPT_max = max(CPT)
    group = [list(range(NCORES))]

    nc = bacc.Bacc("TRN2", num_devices=NCORES, debug=False,
                   dynamic_dma_scratch_size=32768,
                   num_swdge_queues=4)

    # --- I/O ---
    embed_d = nc.dram_tensor("embed", [65, D], f32, kind="ExternalInput")
    w1_d = nc.dram_tensor("w1", [L * D, D], f32, kind="ExternalInput")
    w2_d = nc.dram_tensor("w2", [L * D, D], f32, kind="ExternalInput")
    b1_d = nc.dram_tensor("b1t", [P, L], f32, kind="ExternalInput")
    b2_d = nc.dram_tensor("b2t", [P, L], f32, kind="ExternalInput")
    gm_d = nc.dram_tensor("gammat", [P, L - 1], f32, kind="ExternalInput")
    bt_d = nc.dram_tensor("betat", [P, L - 1], f32, kind="ExternalInput")
    idx_d = nc.dram_tensor("idx", [P, CH * 8], mybir.dt.int16, kind="ExternalInput")
    slot_d = nc.dram_tensor("slotv", [P, CH], f32, kind="ExternalInput")
    degidx_d = nc.dram_tensor("degidx", [P, T], mybir.dt.int32, kind="ExternalInput")
    ct_d = nc.dram_tensor("ct", [64, PADN], f32, kind="ExternalInput")
    cnt_d = nc.dram_tensor("cnt", [1, NCALLS], mybir.dt.int32, kind="ExternalInput")
    out_d = nc.dram_tensor("out", [P, PADN], f32, kind="ExternalOutput")
    if debug_taps:
        dbg_xb = nc.dram_tensor("dbg_xb", [P, PADN], f32, kind="ExternalOutput")
        dbg_st = nc.dram_tensor("dbg_st", [P, 6], f32, kind="ExternalOutput")
        dbg_xa = nc.dram_tensor("dbg_xa", [P, PADN], f32, kind="ExternalOutput")
        dbg_h = nc.dram_tensor("dbg_h", [P, PADN], f32, kind="ExternalOutput")
        dbg_z1 = nc.dram_tensor("dbg_z1", [P, PADN], f32, kind="ExternalOutput")

    with tile.TileContext(nc) as tc:
        with tc.tile_pool(name="persist", bufs=1) as pp, \
             tc.tile_pool(name="gpool", bufs=3) as gp, \
             tc.tile_pool(name="mpool", bufs=3) as mp, \
             tc.tile_pool(name="ipool", bufs=3) as ip, \
             tc.tile_pool(name="small", bufs=2) as sp, \
             tc.tile_pool(name="rowp", bufs=4) as rp, \
             tc.tile_pool(name="agg_ps", bufs=2, space="PSUM") as aggp, \
             tc.tile_pool(name="mlp_ps", bufs=2, space="PSUM") as mlpp, \
             tc.tile_pool(name="tr_ps", bufs=2, space="PSUM") as trp, \
             tc.tile_pool(name="dram", bufs=1, space="DRAM") as dp:

            # --- persistent SBUF state ---
            xA = pp.tile([P, PADN], f32)          # x^T (current layer input)
            xB = pp.tile([P, PADN], f32)          # z2^T (layer output)
            slot_res = pp.tile([P, CH], dt_x)
            deg_res = pp.tile([P, T], mybir.dt.int32)
            w1_sb = pp.tile([P, L * D], f32)
            w2_sb = pp.tile([P, L * D], f32)
            b1_sb = pp.tile([P, L], f32)
            b2_sb = pp.tile([P, L], f32)
            gm_sb = pp.tile([P, L - 1], f32)
            bt_sb = pp.tile([P, L - 1], f32)
            ident = pp.tile([P, P], f32)
            iota_x = pp.tile([P, P], dt_x)
            iota_i = pp.tile([P, P], mybir.dt.int32)
            embed_x = pp.tile([64, D], dt_x)
            cnt_res = pp.tile([1, NCALLS], mybir.dt.int32)
            g_bufs = [pp.tile([P, CPT_max * D], dt_x, name=f"gbuf{i}")
                      for i in range(3)]
            sum_cols = pp.tile([P, T], f32)
            sq_cols = pp.tile([P, T], f32)
            ssum = pp.tile([P, 2], f32)
            gstat = pp.tile([P, 2], f32)
            bn_sc = pp.tile([P, 8], f32)          # scratch columns for BN math

            # --- DRAM buffers ---
            xin = dp.tile([PADN, D], dt_x)
            x_fulls = [dp.tile([XROWS, D], dt_x, addr_space="Shared",
                               name=f"x_full{i}") for i in range(L)]
            st_in = dp.tile([P, 2], f32)
            st_outs = [dp.tile([P, 2], f32, addr_space="Shared",
                               name=f"st_out{i}") for i in range(L - 1)]

            # --- load constants ---
            if dt_x == f32:
                nc.sync.dma_start(out=slot_res[:], in_=slot_d[:])
            else:
                nc.gpsimd.dma_start(out=slot_res[:], in_=slot_d[:])  # f32->bf16 cast
            nc.sync.dma_start(out=deg_res[:], in_=degidx_d[:])
            nc.sync.dma_start(out=w1_sb[:], in_=w1_d[:].rearrange("(l k) m -> k l m", k=P))
            nc.sync.dma_start(out=w2_sb[:], in_=w2_d[:].rearrange("(l k) m -> k l m", k=P))
            nc.sync.dma_start(out=b1_sb[:], in_=b1_d[:])
            nc.sync.dma_start(out=b2_sb[:], in_=b2_d[:])
            nc.sync.dma_start(out=gm_sb[:], in_=gm_d[:])
            nc.sync.dma_start(out=bt_sb[:], in_=bt_d[:])
            make_identity(nc, ident[:])
            nc.sync.dma_start(out=cnt_res[:], in_=cnt_d[:])
            nv_reg = nc.gpsimd.alloc_register("nv")
            for gb in g_bufs:
                nc.vector.memset(gb[:], 0.0)
            g_rot = [0]
            embed_f = pp.tile([64, D], f32)
            nc.sync.dma_start(out=embed_f[:], in_=embed_d[0:64, :])
            nc.vector.tensor_copy(embed_x[:], embed_f[:])
            nc.gpsimd.iota(iota_i[:], pattern=[[1, P]], base=0, channel_multiplier=0)
            nc.vector.tensor_copy(iota_x[:], iota_i[:])

            def iota_bc(K):
                a = iota_x[:]
                return bass.AP(a.tensor, a.offset, [a.ap[0], [0, K], a.ap[1]])

            # --- prologue: x0 = embed[deg], write xin rows + xA (transposed) ---
            for t in range(T):
                rows = rp.tile([P, D], f32, name="rows0")
                nc.gpsimd.indirect_dma_start(
                    out=rows[:], out_offset=None,
                    in_=embed_d[:],
                    in_offset=IndirectOffsetOnAxis(ap=deg_res[:, t:t + 1], axis=0),
                )
                tr = trp.tile([P, P], f32, space="PSUM", name="trt")
                nc.tensor.transpose(out=tr[:], in_=rows[:], identity=ident[:])
                nc.vector.tensor_copy(xA[:, ts(t, P)], tr[:])

            for lyr in range(L):
                # AllGather features for this layer (layer 0 needs none: its
                # aggregation is the C-matrix matmul, not an edge gather)
                x_full = x_fulls[lyr]
                if lyr > 0:
                    nc.gpsimd.collective_compute(
                        "AllGather", mybir.AluOpType.bypass,
                        replica_groups=group,
                        ins=[xin.opt()], outs=[x_full.opt()],
                    )

                last = lyr == L - 1
                epsf = float(1.0 + eps_vals[lyr])
                call_no = [[-1] * NW for _ in range(T)]
                _cn = 0
                for _t in range(T):
                    for _w in range(NW):
                        if K_tw[_t][_w] > 0:
                            call_no[_t][_w] = _cn
                            _cn += 1
                for t in range(T):
                    K = CPT[t]
                    o = off_t[t]
                    h = sp.tile([P, P], f32, name="h")
                    if lyr == 0:
                        # agg0 = (C @ embed)^T: one K=64 matmul per tile
                        ctt = ip.tile([64, P], dt_x, name="ctt")
                        if dt_x == f32:
                            nc.sync.dma_start(out=ctt[:], in_=ct_d[:, ts(t, P)])
                        else:
                            nc.gpsimd.dma_start(out=ctt[:], in_=ct_d[:, ts(t, P)])
                        agg = aggp.tile([P, P], f32, space="PSUM", name="agg")
                        nc.tensor.matmul(out=agg[:], lhsT=embed_x[:], rhs=ctt[:],
                                         start=True, stop=True)
                        nc.vector.scalar_tensor_tensor(
                            out=h[:], in0=xA[:, ts(t, P)], scalar=epsf, in1=agg[:],
                            op0=mybir.AluOpType.mult, op1=mybir.AluOpType.add,
                        )
                        _mlp_tail_marker = True
                    elif K == 0:
                        # all-fake tile: no incoming edges, agg == 0
                        nc.vector.tensor_scalar_mul(h[:], xA[:, ts(t, P)], epsf)
                    else:
                        G = g_bufs[g_rot[0] % 3]
                        g_rot[0] += 1
                        it = ip.tile([P, CPT_max * 8], mybir.dt.int16, name="it")
                        nc.sync.dma_start(out=it[:, :K * 8],
                                          in_=idx_d[:, o * 8:(o + K) * 8])
                        for w in range(NW):
                            Kw = K_tw[t][w]
                            if Kw == 0:
                                continue
                            c0 = off_tw[t][w] - o
                            # queue w -> Q7 core pair (2w, 2w+1); the four
                            # window gathers of a tile generate descriptors
                            # on disjoint core pairs concurrently.
                            nc.gpsimd.dma_gather(
                                out_ap=G[:, c0 * D:(c0 + Kw) * D].rearrange(
                                    "p (k j) -> p k j", j=D),
                                in_ap=x_full[w * W:(w + 1) * W, :],
                                idxs_ap=it[:, c0 * 8:c0 * 8 + NV16[t][w] // 16],
                                num_idxs=NV16[t][w],
                                num_idxs_reg=NV16[t][w],
                                elem_size=D,
                                queue_num=w,
                            )
                        M = mp.tile([P, CPT_max * D], dt_x, name="M")
                        nc.vector.tensor_tensor(
                            out=M[:, :K * D].rearrange("p (k j) -> p k j", j=P),
                            in0=slot_res[:, o:o + K].to_broadcast([P, K, P]),
                            in1=iota_bc(K),
                            op=mybir.AluOpType.is_equal,
                        )
                        agg = aggp.tile([P, P], f32, space="PSUM", name="agg")
                        for k in range(K):
                            nc.tensor.matmul(
                                out=agg[:], lhsT=G[:, ts(k, D)], rhs=M[:, ts(k, D)],
                                start=(k == 0), stop=(k == K - 1),
                            )
                        # h = (1+eps)*x + agg
                        nc.vector.scalar_tensor_tensor(
                            out=h[:], in0=xA[:, ts(t, P)], scalar=epsf, in1=agg[:],
                            op0=mybir.AluOpType.mult, op1=mybir.AluOpType.add,
                        )
                    if debug_taps and lyr == 0:
                        nc.sync.dma_start(out=dbg_h[:, ts(t, P)], in_=h[:])
                    z1p = mlpp.tile([P, P], f32, space="PSUM", name="z1p")
                    nc.tensor.matmul(out=z1p[:], lhsT=w1_sb[:, ts(lyr, D)], rhs=h[:],
                                     start=True, stop=True)
                    # z1 = lrelu(z1p + b1)
                    t1 = sp.tile([P, P], f32, name="t1")
                    nc.scalar.activation(
                        out=t1[:], in_=z1p[:],
                        func=mybir.ActivationFunctionType.Identity,
                        bias=b1_sb[:, lyr:lyr + 1], scale=1.0)
                    z1 = sp.tile([P, P], f32, name="z1")
                    nc.vector.scalar_tensor_tensor(
                        out=z1[:], in0=t1[:], scalar=NEG_SLOPE, in1=t1[:],
                        op0=mybir.AluOpType.mult, op1=mybir.AluOpType.max,
                    )
                    if debug_taps and lyr == 0:
                        nc.sync.dma_start(out=dbg_z1[:, ts(t, P)], in_=z1[:])
                    z2p = mlpp.tile([P, P], f32, space="PSUM", name="z2p")
                    nc.tensor.matmul(out=z2p[:], lhsT=w2_sb[:, ts(lyr, D)], rhs=z1[:],
                                     start=True, stop=True)
                    # xB_tile = z2p + b2 (+ per-tile stats sum)
                    if not last:
                        nc.scalar.activation(
                            out=xB[:, ts(t, P)], in_=z2p[:],
                            func=mybir.ActivationFunctionType.Identity,
                            bias=b2_sb[:, lyr:lyr + 1], scale=1.0,
                            accum_out=sum_cols[:, t:t + 1])
                        sq = sp.tile([P, P], f32, name="sq")
                        nc.vector.tensor_tensor(
                            out=sq[:], in0=xB[:, ts(t, P)], in1=xB[:, ts(t, P)],
                            op=mybir.AluOpType.mult)
                        nc.vector.reduce_sum(out=sq_cols[:, t:t + 1], in_=sq[:],
                                             axis=mybir.AxisListType.X)
                    else:
                        nc.scalar.activation(
                            out=xB[:, ts(t, P)], in_=z2p[:],
                            func=mybir.ActivationFunctionType.Identity,
                            bias=b2_sb[:, lyr:lyr + 1], scale=1.0)

                if last:
                    nc.sync.dma_start(out=out_d[:], in_=xB[:])
                    continue

                # fake-node columns must not pollute the batch statistics
                if PADN > NPC:
                    nc.gpsimd.memset(xB[:, NPC:PADN], 0.0)
                    lt = T - 1
                    nc.scalar.activation(
                        out=xB[:, ts(lt, P)], in_=xB[:, ts(lt, P)],
                        func=mybir.ActivationFunctionType.Identity,
                        bias=0.0, scale=1.0, accum_out=sum_cols[:, lt:lt + 1])
                    sq = sp.tile([P, P], f32, name="sqf")
                    nc.vector.tensor_tensor(
                        out=sq[:], in0=xB[:, ts(lt, P)], in1=xB[:, ts(lt, P)],
                        op=mybir.AluOpType.mult)
                    nc.vector.reduce_sum(out=sq_cols[:, lt:lt + 1], in_=sq[:],
                                         axis=mybir.AxisListType.X)

                if debug_taps and lyr == 0:
                    nc.sync.dma_start(out=dbg_xb[:], in_=xB[:])
                # global BN statistics
                nc.vector.reduce_sum(out=ssum[:, 0:1], in_=sum_cols[:],
                                     axis=mybir.AxisListType.X)
                nc.vector.reduce_sum(out=ssum[:, 1:2], in_=sq_cols[:],
                                     axis=mybir.AxisListType.X)
                nc.sync.dma_start(out=st_in[:], in_=ssum[:])
                st_out = st_outs[lyr]
                nc.gpsimd.collective_compute(
                    "AllReduce", mybir.AluOpType.add,
                    replica_groups=group,
                    ins=[st_in.opt()], outs=[st_out.opt()],
                )
                nc.sync.dma_start(out=gstat[:], in_=st_out[:])
                mu = bn_sc[:, 0:1]; ex2 = bn_sc[:, 1:2]; musq = bn_sc[:, 2:3]
                var = bn_sc[:, 3:4]; std = bn_sc[:, 4:5]; rstd = bn_sc[:, 5:6]
                scl = bn_sc[:, 6:7]; sft = bn_sc[:, 7:8]
                nc.vector.tensor_scalar_mul(mu, gstat[:, 0:1], 1.0 / N)
                nc.vector.tensor_scalar_mul(ex2, gstat[:, 1:2], 1.0 / N)
                nc.vector.tensor_tensor(out=musq, in0=mu, in1=mu,
                                        op=mybir.AluOpType.mult)
                nc.vector.tensor_sub(var, ex2, musq)
                nc.vector.tensor_scalar_add(var, var, BN_EPS)
                nc.scalar.activation(out=std, in_=var,
                                     func=mybir.ActivationFunctionType.Sqrt,
                                     bias=0.0, scale=1.0)
                nc.vector.reciprocal(rstd, std)
                nc.vector.tensor_tensor(out=scl, in0=rstd,
                                        in1=gm_sb[:, lyr:lyr + 1],
                                        op=mybir.AluOpType.mult)
                nc.vector.tensor_tensor(out=musq, in0=mu, in1=scl,
                                        op=mybir.AluOpType.mult)
                nc.vector.tensor_sub(sft, bt_sb[:, lyr:lyr + 1], musq)
                # xA = lrelu(xB*scl + sft); then re-zero fake columns
                nc.scalar.activation(
                    out=xB[:], in_=xB[:],
                    func=mybir.ActivationFunctionType.Identity,
                    bias=sft, scale=scl)
                nc.vector.scalar_tensor_tensor(
                    out=xA[:], in0=xB[:], scalar=NEG_SLOPE, in1=xB[:],
                    op0=mybir.AluOpType.mult, op1=mybir.AluOpType.max,
                )
                if PADN > NPC:
                    nc.gpsimd.memset(xA[:, NPC:PADN], 0.0)
                if debug_taps and lyr == 0:
                    nc.sync.dma_start(out=dbg_st[:, 0:2], in_=ssum[:])
                    nc.sync.dma_start(out=dbg_st[:, 2:4], in_=gstat[:])
                    nc.sync.dma_start(out=dbg_st[:, 4:5], in_=bn_sc[:, 6:7])
                    nc.sync.dma_start(out=dbg_st[:, 5:6], in_=bn_sc[:, 7:8])
                    nc.sync.dma_start(out=dbg_xa[:], in_=xA[:])

                # epilogue: transpose back to rows, refresh xin for next AG
                for t in range(T):
                    tr = trp.tile([P, P], f32, space="PSUM", name="trt")
                    nc.tensor.transpose(out=tr[:], in_=xA[:, ts(t, P)],
                                        identity=ident[:])
                    rows = rp.tile([P, D], dt_x, name="rowse")
                    nc.vector.tensor_copy(rows[:], tr[:])
                    nc.sync.dma_start(out=xin[ts(t, P), :], in_=rows[:])

    nc.compile()
    if legalize:
        import concourse.mybir as mybir2
        _legalize_waits(nc, mybir2)
    return nc


def _host_inputs(inputs, meta, idx_arr, slot_arr, deg_idx, ct, cnt_tab):
    embed = np.zeros((65, D), np.float32)
    embed[:64] = np.asarray(inputs["embed_deg"], np.float32)
    W1 = np.asarray(inputs["W1"], np.float32).reshape(L * D, D)
    W2 = np.asarray(inputs["W2"], np.float32).reshape(L * D, D)
    b1t = np.ascontiguousarray(np.asarray(inputs["b1"], np.float32).T)  # [D, L]
    b2t = np.ascontiguousarray(np.asarray(inputs["b2"], np.float32).T)
    gmt = np.ascontiguousarray(np.asarray(inputs["bn_gamma"], np.float32).T)
    btt = np.ascontiguousarray(np.asarray(inputs["bn_beta"], np.float32).T)
    in_maps = []
    for c in range(NCORES):
        in_maps.append(dict(
            embed=embed, w1=W1, w2=W2, b1t=b1t, b2t=b2t, gammat=gmt, betat=btt,
            idx=idx_arr[c], slotv=slot_arr[c], degidx=deg_idx[c], ct=ct[c],
            cnt=cnt_tab[c],
        ))
    return in_maps


def run_gnn(inputs, n_nodes, dt_x_name=DT_X_NAME, trace=False, tmpdir=None):
    """Build + run the SPMD kernel; returns ([N, D] float32, exec_time_ns)."""
    from concourse.bass_utils import run_bass_kernel_spmd

    node_deg = np.asarray(inputs["node_deg"])
    edge_index = np.asarray(inputs["edge_index"])
    meta, idx_arr, slot_arr, deg_idx, ct, cnt_tab = _prep(node_deg, edge_index, n_nodes)
    eps_vals = [float(e) for e in np.asarray(inputs["eps"], np.float32)]
    nc = _build(meta, eps_vals, dt_x_name)
    in_maps = _host_inputs(inputs, meta, idx_arr, slot_arr, deg_idx, ct, cnt_tab)
    res = run_bass_kernel_spmd(
        nc, in_maps, list(range(NCORES)), trace=trace, tmpdir=tmpdir)
    NPC = meta["NPC"]
    parts = [res.results[c]["out"][:, :NPC] for c in range(NCORES)]
    out = np.concatenate(parts, axis=1).T.astype(np.float32)
    return np.ascontiguousarray(out), res.exec_time_ns


def kernel(**inputs):
    out, _ = run_gnn(inputs, n_nodes=100000)
    return out



# revision 8
# speedup vs baseline: 2.7100x; 1.0809x over previous
"""Trainium2 Bass kernel for a 4-layer GIN GNN (nn_ClassicGNN).

Strategy (graph/data parallel over 8 NeuronCores, dst-sharded):
  - Nodes are block-sharded: core c owns global nodes [c*NPC, (c+1)*NPC),
    padded to PADN = T*128 locally (fake nodes stay zero).
  - Node features live on-chip feature-major: x^T [D=128 partitions, PADN].
  - Per layer: AllGather row-major features into a DRAM buffer x_full
    [NCORES*PADN, D]; edges (pre-sorted by destination on the host and
    packed into 128-edge chunks per 128-node destination tile) drive
    dma_gather row gathers, spread across the 4 SWDGE queues (one Q7 core
    pair each); a one-hot slot matrix M (vector is_equal against an iota)
    turns segment-sum into PSUM matmul accumulation: agg^T = sum_k G_k^T M_k.
  - h = (1+eps)*x + agg fuses as one scalar_tensor_tensor op; the MLP runs
    in bf16 with W1/W2 stationary; BatchNorm statistics are free-dim
    reductions + a [128,2] SBUF->SBUF AllReduce; BN apply + leaky ReLU are
    two elementwise passes; the epilogue transposes tiles back to row-major
    through PSUM (scalar-engine copies) and ships one batched DMA to xin.
  - x0 = embed[node_deg] is computed on the host and loaded via a
    transposing DMA; layer-0 aggregation is the degree-histogram matmul
    agg0^T = embed^T @ C (C built on the host, exact small ints in bf16).

kernel(**inputs) takes the full-size inputs and returns the full [N, D]
output, distributing across 8 cores internally.
"""

import math

import numpy as np

NCORES = 8
P = 128
D = 128
L = 4
NEG_SLOPE = 0.01
BN_EPS = 1e-5


# ---------------------------------------------------------------------------
# Walrus codegen only encodes a single sync wait per instruction; the Tile
# scheduler sometimes attaches more (DMA lane-reuse + data dep, end-of-kernel
# drains). Hoist extras onto same-engine single-wait NoOps placed immediately
# before the instruction (engine streams execute in order, so AND semantics
# are preserved).
# ---------------------------------------------------------------------------
def _legalize_waits(nc, mybir, max_waits=1):
    n_fixed = 0
    for fn in nc.m.functions:
        for bb in fn.blocks:
            insts = bb.instructions
            out = []
            changed = False
            for inst in insts:
                si = inst.sync_info
                if si is not None and len(si.on_wait) > max_waits:
                    waits = list(si.on_wait)
                    keep = waits[len(waits) - max_waits:]
                    hoist = waits[: len(waits) - max_waits]
                    for w in hoist:
                        nop = mybir.InstNoOp(
                            name=nc.get_next_instruction_name(),
                            ins=[],
                            outs=[],
                            engine=inst.engine,
                            debug=inst.debug,
                        )
                        nop.sync_info = mybir.SyncInfo(on_wait=[w], on_update=[])
                        out.append(nop)
                    inst.sync_info = mybir.SyncInfo(
                        on_wait=keep, on_update=list(si.on_update)
                    )
                    n_fixed += 1
                    changed = True
                out.append(inst)
            if changed:
                insts[:] = out
    return n_fixed


# ---------------------------------------------------------------------------
# Host-side preprocessing: shard nodes, sort/pack edges by destination tile.
# ---------------------------------------------------------------------------
def _prep(node_deg, edge_index, n_nodes):
    N = n_nodes
    E = edge_index.shape[1]
    NPC = N // NCORES
    T = math.ceil((NPC + 1) / P)  # always >= 1 fake (zero) row per core
    PADN = T * P
    NW = NCORES // 2              # int16 source windows of 2*PADN rows
    W = 2 * PADN
    assert W <= 32767

    src = edge_index[0].astype(np.int64)
    dst = edge_index[1].astype(np.int64)
    core = dst // NPC
    ld = dst % NPC
    srow = (src // NPC) * PADN + (src % NPC)
    t_of = ld // P
    slot = ld % P
    w_of = srow // W
    lidx = srow - w_of * W

    key = ((core * T + t_of) * NW + w_of)
    order = np.argsort(key, kind="stable")
    ks = key[order]
    lidx_s = lidx[order].astype(np.int16)
    slot_s = slot[order].astype(np.float32)

    counts = np.bincount(key, minlength=NCORES * T * NW)
    starts = np.zeros(NCORES * T * NW + 1, np.int64)
    starts[1:] = np.cumsum(counts)
    rank = np.arange(E, dtype=np.int64) - starts[ks]

    cnt = counts.reshape(NCORES, T, NW)
    K_tw = ((cnt.max(axis=0) + P - 1) // P).astype(np.int64)   # [T, NW]
    # per-call descriptor count: round the max core count to the 16-index
    # wrapping granularity (not 128) so Q7 skips most pad descriptors
    NV16 = (np.maximum(cnt.max(axis=0), 1) + 15) // 16 * 16    # [T, NW]
    CPT = K_tw.sum(axis=1).astype(np.int64)                    # chunks per tile
    off_tw = np.zeros(T * NW + 1, np.int64)
    off_tw[1:] = np.cumsum(K_tw.ravel())
    off_tw = off_tw[:-1].reshape(T, NW)                        # global chunk col
    off_t = np.zeros(T + 1, np.int64)
    off_t[1:] = np.cumsum(CPT)
    CH = int(off_t[-1])

    # pads are negative indices (the Q7 loop trims the trailing negatives)
    # and carry slot=-1 so the one-hot matrix zeroes whatever stale data sits
    # in the skipped G positions. Each (t,w) call keeps at least one valid
    # index (row 0) so the gather is never empty.
    idx16 = np.zeros((NCORES, P, CH), np.int16)
    slot_arr = np.full((NCORES, P, CH), -1.0, np.float32)
    tw = ks % (T * NW)
    col = off_tw.ravel()[tw] + rank // P
    part = rank % P
    idx16[ks // (T * NW), part, col] = lidx_s
    slot_arr[ks // (T * NW), part, col] = slot_s
    # empty (core,t,w) groups with K_tw>0 get one valid dummy index
    for c in range(NCORES):
        empty = (cnt[c] == 0) & (K_tw > 0)
        for t0, w0 in zip(*np.nonzero(empty)):
            idx16[c, 0, off_tw[t0, w0]] = 0

    # dma_gather wrapped index layout: flat index i -> partition i%16,
    # column i//16, replicated over the 8 gpsimd cores (16-partition blocks).
    a = idx16.reshape(NCORES, 8, 16, CH)        # p = s*16 + r -> (s, r)
    wr = a.transpose(0, 2, 3, 1).reshape(NCORES, 16, CH * 8)
    widx = np.ascontiguousarray(np.tile(wr, (1, 8, 1)))  # [NC, 128, CH*8]

    # layer-0 aggregation is a degree histogram: agg0[n] = sum_k C[n,k] embed[k]
    # (x0 = embed[node_deg] has only 64 distinct rows). C is exact small ints.
    sdeg = node_deg.astype(np.int64)[src]
    Cfull = np.bincount(dst * 64 + sdeg, minlength=N * 64).reshape(N, 64)
    ct = np.zeros((NCORES, 64, PADN), np.float32)
    for c in range(NCORES):
        ct[c, :, :NPC] = Cfull[c * NPC:(c + 1) * NPC].T

    meta = dict(N=N, NPC=NPC, T=T, PADN=PADN, CH=CH, NW=NW, W=W,
                K_tw=[[int(k) for k in row] for row in K_tw],
                NV16=[[int(v) for v in row] for row in NV16],
                CPT=[int(c) for c in CPT],
                off_tw=[[int(o) for o in row] for row in off_tw],
                off_t=[int(o) for o in off_t])
    return meta, widx, slot_arr, ct


# ---------------------------------------------------------------------------
# Bass/Tile program
# ---------------------------------------------------------------------------
def _build(meta, eps_vals, legalize=True):
    import concourse.bass as bass
    import concourse.bacc as bacc
    import concourse.mybir as mybir
    import concourse.tile as tile
    from concourse.bass import ts
    from concourse.masks import make_identity

    # The end-of-kernel semaphore RANGE_CLEAR chokes walrus codegen ("ISA
    # wrong length") when the range is large; clear in small chunks instead.
    if not getattr(bass.Bass, "_cafs_chunked", False):
        _orig_cafs = bass.Bass.clear_and_free_semaphores

        def _chunked(self, sems, _orig=_orig_cafs):
            sems = list(sems)
            for i in range(0, len(sems), 8):
                _orig(self, sems[i:i + 8])

        bass.Bass.clear_and_free_semaphores = _chunked
        bass.Bass._cafs_chunked = True

    f32 = mybir.dt.float32
    bf16 = mybir.dt.bfloat16
    T, PADN, CH, NPC = meta["T"], meta["PADN"], meta["CH"], meta["NPC"]
    NW, W = meta["NW"], meta["W"]
    K_tw, CPT, off_tw, off_t = meta["K_tw"], meta["CPT"], meta["off_tw"], meta["off_t"]
    NV16 = meta["NV16"]
    N = meta["N"]
    XROWS = NCORES * PADN
    CPT_max = max(CPT)
    group = [list(range(NCORES))]

    nc = bacc.Bacc("TRN2", num_devices=NCORES, debug=False,
                   dynamic_dma_scratch_size=32768,
                   num_swdge_queues=4)

    # --- I/O ---
    x0_d = nc.dram_tensor("x0", [PADN, D], bf16, kind="ExternalInput")
    w1_d = nc.dram_tensor("w1", [L * D, D], bf16, kind="ExternalInput")
    w2_d = nc.dram_tensor("w2", [L * D, D], bf16, kind="ExternalInput")
    b1_d = nc.dram_tensor("b1t", [P, L], f32, kind="ExternalInput")
    b2_d = nc.dram_tensor("b2t", [P, L], f32, kind="ExternalInput")
    gm_d = nc.dram_tensor("gammat", [P, L - 1], f32, kind="ExternalInput")
    bt_d = nc.dram_tensor("betat", [P, L - 1], f32, kind="ExternalInput")
    idx_d = nc.dram_tensor("idx", [P, CH * 8], mybir.dt.int16, kind="ExternalInput")
    slot_d = nc.dram_tensor("slotv", [P, CH], bf16, kind="ExternalInput")
    embed_d = nc.dram_tensor("embed", [64, D], bf16, kind="ExternalInput")
    ct_d = nc.dram_tensor("ct", [64, PADN], bf16, kind="ExternalInput")
    out_d = nc.dram_tensor("out", [P, PADN], f32, kind="ExternalOutput")

    with tile.TileContext(nc) as tc:
        with tc.tile_pool(name="persist", bufs=1) as pp, \
             tc.tile_pool(name="mpool", bufs=3) as mp, \
             tc.tile_pool(name="ipool", bufs=3) as ip, \
             tc.tile_pool(name="small", bufs=2) as sp, \
             tc.tile_pool(name="rowp", bufs=4) as rp, \
             tc.tile_pool(name="agg_ps", bufs=2, space="PSUM") as aggp, \
             tc.tile_pool(name="mlp_ps", bufs=2, space="PSUM") as mlpp, \
             tc.tile_pool(name="tr_ps", bufs=2, space="PSUM") as trp, \
             tc.tile_pool(name="dram", bufs=1, space="DRAM") as dp:

            # --- persistent SBUF state ---
            xA = pp.tile([P, PADN], bf16)         # x^T (current layer input)
            xB = pp.tile([P, PADN], bf16)         # z2^T (layer output)
            rows_all = pp.tile([P, T * D], bf16)  # row-major epilogue staging
            slot_res = pp.tile([P, CH], bf16)
            w1_sb = pp.tile([P, L * D], bf16)
            w2_sb = pp.tile([P, L * D], bf16)
            b1_sb = pp.tile([P, L], f32)
            b2_sb = pp.tile([P, L], f32)
            gm_sb = pp.tile([P, L - 1], f32)
            bt_sb = pp.tile([P, L - 1], f32)
            ident = pp.tile([P, P], f32)
            ident_x = pp.tile([P, P], bf16)
            iota_x = pp.tile([P, P], bf16)
            iota_i = pp.tile([P, P], mybir.dt.int32)
            embed_x = pp.tile([64, D], bf16)
            g_bufs = [pp.tile([P, CPT_max * D], bf16, name=f"gbuf{i}")
                      for i in range(3)]
            sum_cols = pp.tile([P, T], f32)
            sq_cols = pp.tile([P, T], f32)
            ssum = pp.tile([P, 2], f32)
            gstat = pp.tile([P, 2], f32)
            bn_sc = pp.tile([P, 8], f32)          # scratch columns for BN math

            # --- DRAM buffers ---
            xin = dp.tile([PADN, D], bf16)
            x_fulls = [dp.tile([XROWS, D], bf16, addr_space="Shared",
                               name=f"x_full{i}") for i in range(L - 1)]
            st_in = dp.tile([P, 2], f32)
            st_outs = [dp.tile([P, 2], f32, addr_space="Shared",
                               name=f"st_out{i}") for i in range(L - 1)]

            # --- load constants ---
            nc.sync.dma_start(out=slot_res[:], in_=slot_d[:])
            nc.sync.dma_start(out=w1_sb[:], in_=w1_d[:].rearrange("(l k) m -> k l m", k=P))
            nc.sync.dma_start(out=w2_sb[:], in_=w2_d[:].rearrange("(l k) m -> k l m", k=P))
            nc.sync.dma_start(out=b1_sb[:], in_=b1_d[:])
            nc.sync.dma_start(out=b2_sb[:], in_=b2_d[:])
            nc.sync.dma_start(out=gm_sb[:], in_=gm_d[:])
            nc.sync.dma_start(out=bt_sb[:], in_=bt_d[:])
            nc.sync.dma_start(out=embed_x[:], in_=embed_d[:])
            make_identity(nc, ident[:])
            nc.vector.tensor_copy(ident_x[:], ident[:])
            nc.gpsimd.iota(iota_i[:], pattern=[[1, P]], base=0, channel_multiplier=0)
            nc.vector.tensor_copy(iota_x[:], iota_i[:])
            for gb in g_bufs:
                nc.vector.memset(gb[:], 0.0)

            # x0^T via transposing DMA (bf16), straight into xA
            nc.sync.dma_start(out=xA[:], in_=x0_d[:], transpose=True)

            def iota_bc(K):
                a = iota_x[:]
                return bass.AP(a.tensor, a.offset, [a.ap[0], [0, K], a.ap[1]])

            for lyr in range(L):
                # AllGather features for this layer (layer 0 needs none: its
                # aggregation is the C-matrix matmul, not an edge gather)
                if lyr > 0:
                    x_full = x_fulls[lyr - 1]
                    nc.gpsimd.collective_compute(
                        "AllGather", mybir.AluOpType.bypass,
                        replica_groups=group,
                        ins=[xin.opt()], outs=[x_full.opt()],
                    )

                last = lyr == L - 1
                epsf = float(1.0 + eps_vals[lyr])
                for t in range(T):
                    K = CPT[t]
                    o = off_t[t]
                    h = sp.tile([P, P], bf16, name="h")
                    if lyr == 0:
                        # agg0 = (C @ embed)^T: one K=64 matmul per tile
                        ctt = ip.tile([64, P], bf16, name="ctt")
                        nc.sync.dma_start(out=ctt[:], in_=ct_d[:, ts(t, P)])
                        agg = aggp.tile([P, P], f32, space="PSUM", name="agg")
                        nc.tensor.matmul(out=agg[:], lhsT=embed_x[:], rhs=ctt[:],
                                         start=True, stop=True)
                        nc.vector.scalar_tensor_tensor(
                            out=h[:], in0=xA[:, ts(t, P)], scalar=epsf, in1=agg[:],
                            op0=mybir.AluOpType.mult, op1=mybir.AluOpType.add,
                        )
                    else:
                        G = g_bufs[t % 3]
                        it = ip.tile([P, CPT_max * 8], mybir.dt.int16, name="it")
                        nc.sync.dma_start(out=it[:, :K * 8],
                                          in_=idx_d[:, o * 8:(o + K) * 8])
                        for w in range(NW):
                            Kw = K_tw[t][w]
                            if Kw == 0:
                                continue
                            c0 = off_tw[t][w] - o
                            # queue w -> Q7 core pair (2w, 2w+1); the four
                            # window gathers of a tile generate descriptors
                            # on disjoint core pairs concurrently.
                            nc.gpsimd.dma_gather(
                                out_ap=G[:, c0 * D:(c0 + Kw) * D].rearrange(
                                    "p (k j) -> p k j", j=D),
                                in_ap=x_full[w * W:(w + 1) * W, :],
                                idxs_ap=it[:, c0 * 8:c0 * 8 + NV16[t][w] // 16],
                                num_idxs=NV16[t][w],
                                num_idxs_reg=NV16[t][w],
                                elem_size=D,
                                queue_num=w,
                            )
                        M = mp.tile([P, CPT_max * D], bf16, name="M")
                        nc.vector.tensor_tensor(
                            out=M[:, :K * D].rearrange("p (k j) -> p k j", j=P),
                            in0=slot_res[:, o:o + K].to_broadcast([P, K, P]),
                            in1=iota_bc(K),
                            op=mybir.AluOpType.is_equal,
                        )
                        agg = aggp.tile([P, P], f32, space="PSUM", name="agg")
                        for k in range(K):
                            nc.tensor.matmul(
                                out=agg[:], lhsT=G[:, ts(k, D)], rhs=M[:, ts(k, D)],
                                start=(k == 0), stop=(k == K - 1),
                            )
                        # h = (1+eps)*x + agg
                        nc.vector.scalar_tensor_tensor(
                            out=h[:], in0=xA[:, ts(t, P)], scalar=epsf, in1=agg[:],
                            op0=mybir.AluOpType.mult, op1=mybir.AluOpType.add,
                        )
                    z1p = mlpp.tile([P, P], f32, space="PSUM", name="z1p")
                    nc.tensor.matmul(out=z1p[:], lhsT=w1_sb[:, ts(lyr, D)], rhs=h[:],
                                     start=True, stop=True)
                    # z1 = lrelu(z1p + b1)
                    t1 = sp.tile([P, P], bf16, name="t1")
                    nc.scalar.activation(
                        out=t1[:], in_=z1p[:],
                        func=mybir.ActivationFunctionType.Identity,
                        bias=b1_sb[:, lyr:lyr + 1], scale=1.0)
                    z1 = sp.tile([P, P], bf16, name="z1")
                    nc.vector.scalar_tensor_tensor(
                        out=z1[:], in0=t1[:], scalar=NEG_SLOPE, in1=t1[:],
                        op0=mybir.AluOpType.mult, op1=mybir.AluOpType.max,
                    )
                    z2p = mlpp.tile([P, P], f32, space="PSUM", name="z2p")
                    nc.tensor.matmul(out=z2p[:], lhsT=w2_sb[:, ts(lyr, D)], rhs=z1[:],
                                     start=True, stop=True)
                    if last:
                        # final output tile straight to DRAM in f32
                        of = sp.tile([P, P], f32, name="of")
                        nc.scalar.activation(
                            out=of[:], in_=z2p[:],
                            func=mybir.ActivationFunctionType.Identity,
                            bias=b2_sb[:, lyr:lyr + 1], scale=1.0)
                        nc.sync.dma_start(out=out_d[:, ts(t, P)], in_=of[:])
                        continue
                    # xB_tile = z2p + b2 (+ per-tile stats sum)
                    if t == T - 1:
                        nc.scalar.activation(
                            out=xB[:, ts(t, P)], in_=z2p[:],
                            func=mybir.ActivationFunctionType.Identity,
                            bias=b2_sb[:, lyr:lyr + 1], scale=1.0)
                        # fake-node columns must not pollute batch statistics
                        nc.vector.memset(xB[:, NPC:PADN], 0.0)
                        nc.scalar.activation(
                            out=xB[:, ts(t, P)], in_=xB[:, ts(t, P)],
                            func=mybir.ActivationFunctionType.Identity,
                            bias=0.0, scale=1.0, accum_out=sum_cols[:, t:t + 1])
                    else:
                        nc.scalar.activation(
                            out=xB[:, ts(t, P)], in_=z2p[:],
                            func=mybir.ActivationFunctionType.Identity,
                            bias=b2_sb[:, lyr:lyr + 1], scale=1.0,
                            accum_out=sum_cols[:, t:t + 1])
                    sq = sp.tile([P, P], f32, name="sq")
                    nc.vector.tensor_tensor(
                        out=sq[:], in0=xB[:, ts(t, P)], in1=xB[:, ts(t, P)],
                        op=mybir.AluOpType.mult)
                    nc.vector.reduce_sum(out=sq_cols[:, t:t + 1], in_=sq[:],
                                         axis=mybir.AxisListType.X)

                if last:
                    continue

                # global BN statistics: [128, 2] AllReduce via DRAM
                nc.vector.reduce_sum(out=ssum[:, 0:1], in_=sum_cols[:],
                                     axis=mybir.AxisListType.X)
                nc.vector.reduce_sum(out=ssum[:, 1:2], in_=sq_cols[:],
                                     axis=mybir.AxisListType.X)
                nc.sync.dma_start(out=st_in[:], in_=ssum[:])
                st_out = st_outs[lyr]
                nc.gpsimd.collective_compute(
                    "AllReduce", mybir.AluOpType.add,
                    replica_groups=group,
                    ins=[st_in.opt()], outs=[st_out.opt()],
                )
                nc.sync.dma_start(out=gstat[:], in_=st_out[:])
                mu = bn_sc[:, 0:1]; ex2 = bn_sc[:, 1:2]; musq = bn_sc[:, 2:3]
                var = bn_sc[:, 3:4]; std = bn_sc[:, 4:5]; rstd = bn_sc[:, 5:6]
                scl = bn_sc[:, 6:7]; sft = bn_sc[:, 7:8]
                nc.vector.tensor_scalar_mul(mu, gstat[:, 0:1], 1.0 / N)
                nc.vector.tensor_scalar_mul(ex2, gstat[:, 1:2], 1.0 / N)
                nc.vector.tensor_tensor(out=musq, in0=mu, in1=mu,
                                        op=mybir.AluOpType.mult)
                nc.vector.tensor_sub(var, ex2, musq)
                nc.vector.tensor_scalar_add(var, var, BN_EPS)
                nc.scalar.activation(out=std, in_=var,
                                     func=mybir.ActivationFunctionType.Sqrt,
                                     bias=0.0, scale=1.0)
                nc.vector.reciprocal(rstd, std)
                nc.vector.tensor_tensor(out=scl, in0=rstd,
                                        in1=gm_sb[:, lyr:lyr + 1],
                                        op=mybir.AluOpType.mult)
                nc.vector.tensor_tensor(out=musq, in0=mu, in1=scl,
                                        op=mybir.AluOpType.mult)
                nc.vector.tensor_sub(sft, bt_sb[:, lyr:lyr + 1], musq)
                # xA = lrelu(xB*scl + sft), in 4 column chunks so the next
                # layer's first tiles unblock early; then re-zero fakes via
                # chunked processing order (fakes live in the last chunk).
                CHN = PADN // 4
                for ci in range(4):
                    cs = slice(ci * CHN, (ci + 1) * CHN)
                    nc.scalar.activation(
                        out=xB[:, cs], in_=xB[:, cs],
                        func=mybir.ActivationFunctionType.Identity,
                        bias=sft, scale=scl)
                    nc.vector.scalar_tensor_tensor(
                        out=xA[:, cs], in0=xB[:, cs], scalar=NEG_SLOPE,
                        in1=xB[:, cs],
                        op0=mybir.AluOpType.mult, op1=mybir.AluOpType.max,
                    )
                if PADN > NPC:
                    nc.vector.memset(xA[:, NPC:PADN], 0.0)

                # epilogue: transpose back to rows (PE, bf16), copy PSUM->SBUF
                # on the scalar engine, one batched DMA refreshes xin.
                for t in range(T):
                    tr = trp.tile([P, P], bf16, space="PSUM", name="trt")
                    nc.tensor.transpose(out=tr[:], in_=xA[:, ts(t, P)],
                                        identity=ident_x[:])
                    nc.scalar.activation(
                        out=rows_all[:, ts(t, D)], in_=tr[:],
                        func=mybir.ActivationFunctionType.Identity,
                        bias=0.0, scale=1.0)
                nc.sync.dma_start(
                    out=xin[:].rearrange("(t p) d -> p t d", p=P),
                    in_=rows_all[:].rearrange("p (t d) -> p t d", d=D))

    nc.compile()
    if legalize:
        import concourse.mybir as mybir2
        _legalize_waits(nc, mybir2)
    return nc


def _host_inputs(inputs, meta, idx_arr, slot_arr, ct):
    import ml_dtypes
    bf = ml_dtypes.bfloat16
    NPC, PADN = meta["NPC"], meta["PADN"]
    embed = np.asarray(inputs["embed_deg"], np.float32)
    node_deg = np.asarray(inputs["node_deg"])
    x0 = embed[node_deg]                       # [N, D] host embedding lookup
    W1 = np.asarray(inputs["W1"], np.float32).reshape(L * D, D).astype(bf)
    W2 = np.asarray(inputs["W2"], np.float32).reshape(L * D, D).astype(bf)
    b1t = np.ascontiguousarray(np.asarray(inputs["b1"], np.float32).T)  # [D, L]
    b2t = np.ascontiguousarray(np.asarray(inputs["b2"], np.float32).T)
    gmt = np.ascontiguousarray(np.asarray(inputs["bn_gamma"], np.float32).T)
    btt = np.ascontiguousarray(np.asarray(inputs["bn_beta"], np.float32).T)
    embed_bf = embed.astype(bf)
    in_maps = []
    for c in range(NCORES):
        x0c = np.zeros((PADN, D), bf)
        x0c[:NPC] = x0[c * NPC:(c + 1) * NPC].astype(bf)
        in_maps.append(dict(
            x0=x0c, w1=W1, w2=W2, b1t=b1t, b2t=b2t, gammat=gmt, betat=btt,
            idx=idx_arr[c], slotv=slot_arr[c].astype(bf), embed=embed_bf,
            ct=ct[c].astype(bf),
        ))
    return in_maps


def run_gnn(inputs, n_nodes, trace=False, tmpdir=None):
    """Build + run the SPMD kernel; returns ([N, D] float32, exec_time_ns)."""
    from concourse.bass_utils import run_bass_kernel_spmd

    node_deg = np.asarray(inputs["node_deg"])
    edge_index = np.asarray(inputs["edge_index"])
    meta, idx_arr, slot_arr, ct = _prep(node_deg, edge_index, n_nodes)
    eps_vals = [float(e) for e in np.asarray(inputs["eps"], np.float32)]
    nc = _build(meta, eps_vals)
    in_maps = _host_inputs(inputs, meta, idx_arr, slot_arr, ct)
    res = run_bass_kernel_spmd(
        nc, in_maps, list(range(NCORES)), trace=trace, tmpdir=tmpdir)
    NPC = meta["NPC"]
    parts = [res.results[c]["out"][:, :NPC] for c in range(NCORES)]
    out = np.concatenate(parts, axis=1).T.astype(np.float32)
    return np.ascontiguousarray(out), res.exec_time_ns


def kernel(**inputs):
    out, _ = run_gnn(inputs, n_nodes=100000)
    return out
